# revision 7
# baseline (speedup 1.0000x reference)
"""CLIPMutationLoss forward on 8 Trainium2 NeuronCores (data-parallel over batch).

Per core b: scores[m, t] = logit_scale * dot(text[b*20+m, t, :], gnn[b, coords[b, t], :])
loss = mean_b( sum_t mask*CE0(scores) / sum_t mask ),  acc = global masked argmax==0 rate.

Device pipeline (per core, all heavy data bf16):
  - text slab host-cast to bf16, laid out [4 chunks, 128 part, 2 d-halves, 20 m, 256 t]:
    one contiguous 2.62 MB DMA per chunk (10.5 MB total).
  - gather gnn[coords] WITHOUT indirect DMA: one-hot matmul on PE.
      O_nt[p, t] = (coords[t] == nt*128 + p)   (16 DVE tensor_scalar is_equal ops, fp16)
      selT[h][d, t] = sum_nt gnn_tile[nt].T @ O_nt   (64 bf16 matmuls, fp32 PSUM -> exact)
    This lands sel directly in [d, t] layout — no transposes, no gpsimd descriptors.
  - DVE: P[h] = textT_tile * selT_bcast  (bf16 2x mode; d on partitions, (m, t) on free)
  - PE: scores = ones-vector matmul reduction over d, both halves accumulated into
    PSUM [128 t-in-tile, 160 = (8 tt) x (20 m)] columns.
  - ACT/DVE epilogue (fp32): scale by logit_scale, log-softmax over m, class-0 CE,
    argmax==0 (exact: correct <=> s0 >= max), masked sums, partition-reduce matmul.
  - Output per core: [loss_masked_sum, correct_masked_sum, mask_sum, 0]; host combines.

bf16 error was validated against the exact seeded inputs: loss rel err ~5e-5 and the
masked accuracy is bit-identical.
"""

import numpy as np

import concourse.bacc as bacc
import concourse.bass as bass
import concourse.tile as tile
from concourse import mybir
from concourse.bass_interp import get_hw_module
from concourse.bass_utils import run_bass_kernel_spmd

B, N_NODES, D = 8, 2048, 256
T = 1024
M1 = 20  # num_mutations + 1 classes
NCORES = 8
P = 128
NCH = 4            # token chunks per core
CHT = T // NCH     # 256 tokens per chunk
NT = T // P        # 8 token tiles of 128
NH = D // P        # 2 d-halves
NNT = N_NODES // P  # 16 gnn node tiles
TH = 512           # selT build granularity (max moving free dim)
F32 = mybir.dt.float32
F16 = mybir.dt.float16
BF16 = mybir.dt.bfloat16
I32 = mybir.dt.int32
NP_BF16 = mybir.dt.np(BF16)

_NC_CACHE = {}
LAST_RESULTS = None  # test harness reads exec_time_ns off this


def _build_nc():
    nc = bacc.Bacc("TRN2", target_bir_lowering=False, debug=False)
    textT = nc.dram_tensor("textT", [NCH, P, NH, M1, CHT], BF16, kind="ExternalInput").ap()
    gnn = nc.dram_tensor("gnn", [N_NODES, D], BF16, kind="ExternalInput").ap()
    coords = nc.dram_tensor("coords", [T], F32, kind="ExternalInput").ap()
    maskf = nc.dram_tensor("maskf", [P, NT], F32, kind="ExternalInput").ap()
    ls = nc.dram_tensor("ls", [P, 1], F32, kind="ExternalInput").ap()
    out = nc.dram_tensor("out", [4, 1], F32, kind="ExternalOutput").ap()

    with (
        tile.TileContext(nc) as tc,
        tc.tile_pool(name="consts", bufs=1) as consts,
        tc.tile_pool(name="textp", bufs=3) as textp,
        tc.tile_pool(name="pp", bufs=4) as pp,
        tc.tile_pool(name="soft", bufs=1) as soft,
        tc.tile_pool(name="ps", bufs=1, space="PSUM") as ps,
        tc.tile_pool(name="gps", bufs=2, space="PSUM") as gps,
    ):
        ones_bf = consts.tile([P, 1], BF16)
        nc.vector.memset(ones_bf[:], 1.0)
        ones_f = consts.tile([P, 1], F32)
        nc.vector.memset(ones_f[:], 1.0)
        maskf_sb = consts.tile([P, NT], F32)
        nc.sync.dma_start(out=maskf_sb[:], in_=maskf[:])
        ls_sb = consts.tile([P, 1], F32)
        nc.sync.dma_start(out=ls_sb[:], in_=ls[:])

        # Touch Exp/Ln once at kernel start so the ACT table load (~2.7us) hides
        # under the initial text DMAs instead of landing on the critical tail.
        dummy = consts.tile([P, 1], F32)
        nc.scalar.activation(out=dummy[:], in_=ones_f[:], func=mybir.ActivationFunctionType.Exp)
        nc.scalar.activation(out=dummy[:], in_=dummy[:], func=mybir.ActivationFunctionType.Ln)

        # ---- one-hot gather of gnn rows on PE ----
        # coords broadcast to every partition; gnn/coords ride the SWDGE ring so
        # the HWDGE rings stream text from t=0.
        coords_all = consts.tile([P, T], F32)
        coords_b = bass.AP(tensor=coords.tensor, offset=coords.offset, ap=[[0, P], coords.ap[0]])
        nc.gpsimd.dma_start(out=coords_all[:], in_=coords_b)
        # gnn as 16 node tiles: gnn_sb[p, nt, d] = gnn[nt*128 + p, d]
        gnn_sb = consts.tile([P, NNT, D], BF16)
        nc.gpsimd.dma_start(
            out=gnn_sb[:],
            in_=gnn.rearrange("(nt p) d -> p nt d", p=P),
        )
        # nvals[p, nt] = nt*128 + p
        nvals_i = consts.tile([P, NNT], I32)
        nc.gpsimd.iota(nvals_i[:], pattern=[[P, NNT]], base=0, channel_multiplier=1)
        nvals = consts.tile([P, NNT], F32)
        nc.vector.tensor_copy(out=nvals[:], in_=nvals_i[:])
        # O[p, nt, t] = (coords[t] == nt*128 + p), bf16 one-hot. Built per th-half
        # (tokens 0-511 first) and split across DVE + GpSimd to shorten the head.
        o_all = consts.tile([P, NNT, T], BF16)
        selT = [consts.tile([P, T], BF16, name=f"selT{h}") for h in range(NH)]
        for th in range(T // TH):
            tsl = slice(th * TH, (th + 1) * TH)
            for nt in range(NNT):
                eng = nc.vector if nt % 2 == 0 else nc.gpsimd
                eng.tensor_scalar(
                    out=o_all[:, nt, tsl],
                    in0=coords_all[:, tsl],
                    scalar1=nvals[:, nt : nt + 1],
                    scalar2=None,
                    op0=mybir.AluOpType.is_equal,
                )
            # selT[h][d, t] = gnn[coords[t], h*128 + d] (exact bf16 via fp32 PSUM)
            for h in range(NH):
                sel_ps = gps.tile([P, TH], F32, name="sel_ps")
                for nt in range(NNT):
                    nc.tensor.matmul(
                        out=sel_ps[:],
                        lhsT=gnn_sb[:, nt, h * P : (h + 1) * P],
                        rhs=o_all[:, nt, tsl],
                        start=(nt == 0),
                        stop=(nt == NNT - 1),
                    )
                nc.scalar.copy(out=selT[h][:, tsl], in_=sel_ps[:])

        # ---- per-token class scores ----
        # columns: col = tt*20 + m  (tt = c*2 + tl, token = tt*128 + p)
        scores_ps = ps.tile([P, NT * M1], F32, name="scores_ps")
        for c in range(NCH):
            tx = textp.tile([P, NH, M1, CHT], BF16, name="tx")
            dma_eng = nc.sync if c % 2 == 0 else nc.scalar
            dma_eng.dma_start(out=tx[:], in_=textT[c])
            ptiles = []
            for h in range(NH):
                pt = pp.tile([P, M1, CHT], BF16, name="pt")
                sl = selT[h][:, c * CHT : (c + 1) * CHT]
                sl_b = bass.AP(tensor=sl.tensor, offset=sl.offset, ap=[sl.ap[0], [0, M1], sl.ap[1]])
                nc.vector.tensor_tensor(out=pt[:], in0=tx[:, h], in1=sl_b, op=mybir.AluOpType.mult)
                ptiles.append(pt)
            for g in range(2 * M1):
                tl, m = divmod(g, M1)
                col = c * 2 * M1 + g
                for h in range(NH):
                    nc.tensor.matmul(
                        out=scores_ps[:, col : col + 1],
                        lhsT=ptiles[h][:, m, tl * P : (tl + 1) * P],
                        rhs=ones_bf[:],
                        start=(h == 0),
                        stop=(h == NH - 1),
                    )

        # ---- epilogue: log-softmax over m, class-0 CE, accuracy, masked sums ----
        scores_sb = soft.tile([P, NT, M1], F32)
        nc.scalar.activation(
            out=scores_sb[:].rearrange("p t m -> p (t m)"),
            in_=scores_ps[:],
            func=mybir.ActivationFunctionType.Copy,
            bias=0.0,
            scale=ls_sb[:, 0:1],
        )
        mx = soft.tile([P, NT], F32)
        nc.vector.reduce_max(out=mx[:], in_=scores_sb[:], axis=mybir.AxisListType.X)
        sub = soft.tile([P, NT, M1], F32)
        mx_b = bass.AP(tensor=mx.tensor, offset=mx[:].offset, ap=[mx[:].ap[0], [1, NT], [0, M1]])
        nc.vector.tensor_tensor(out=sub[:], in0=scores_sb[:], in1=mx_b, op=mybir.AluOpType.subtract)
        expt = soft.tile([P, NT, M1], F32)
        nc.scalar.activation(out=expt[:], in_=sub[:], func=mybir.ActivationFunctionType.Exp)
        se = soft.tile([P, NT], F32)
        nc.vector.reduce_sum(out=se[:], in_=expt[:], axis=mybir.AxisListType.X)
        lse = soft.tile([P, NT], F32)
        nc.scalar.activation(out=lse[:], in_=se[:], func=mybir.ActivationFunctionType.Ln)

        s0 = bass.AP(tensor=scores_sb.tensor, offset=scores_sb[:].offset, ap=[scores_sb[:].ap[0], [M1, NT]])
        tmp = soft.tile([P, NT], F32)
        nc.vector.tensor_add(out=tmp[:], in0=mx[:], in1=lse[:])
        ltok = soft.tile([P, NT], F32)
        nc.vector.tensor_tensor(out=ltok[:], in0=tmp[:], in1=s0, op=mybir.AluOpType.subtract)
        corr = soft.tile([P, NT], F32)
        nc.vector.tensor_tensor(out=corr[:], in0=s0, in1=mx[:], op=mybir.AluOpType.is_ge)

        ml = soft.tile([P, NT], F32)
        nc.vector.tensor_mul(out=ml[:], in0=ltok[:], in1=maskf_sb[:])
        mc = soft.tile([P, NT], F32)
        nc.vector.tensor_mul(out=mc[:], in0=corr[:], in1=maskf_sb[:])

        stats = soft.tile([P, 4], F32)
        nc.vector.memset(stats[:], 0.0)
        nc.vector.reduce_sum(out=stats[:, 0:1], in_=ml[:], axis=mybir.AxisListType.X)
        nc.vector.reduce_sum(out=stats[:, 1:2], in_=mc[:], axis=mybir.AxisListType.X)
        nc.vector.reduce_sum(out=stats[:, 2:3], in_=maskf_sb[:], axis=mybir.AxisListType.X)

        stat_ps = ps.tile([4, 1], F32, name="stat_ps")
        nc.tensor.matmul(out=stat_ps[:], lhsT=stats[:], rhs=ones_f[:], start=True, stop=True)
        out_sb = soft.tile([4, 1], F32)
        nc.scalar.copy(out=out_sb[:], in_=stat_ps[:])
        nc.sync.dma_start(out=out[:], in_=out_sb[:])

    nc.compile()
    nc.m = get_hw_module(nc.m)
    return nc


def get_nc():
    if "nc" not in _NC_CACHE:
        _NC_CACHE["nc"] = _build_nc()
    return _NC_CACHE["nc"]


def make_in_maps(gnn_features, text_features, logit_scale, seq_to_coords, seq_loss_mask):
    in_maps = []
    lsv = np.float32(np.asarray(logit_scale).reshape(-1)[0])
    for b in range(NCORES):
        slab = np.asarray(text_features[b * M1 : (b + 1) * M1], dtype=np.float32)  # [20, 1024, 256]
        tT = slab.transpose(2, 0, 1)                      # [256 d, 20 m, 1024 t]
        tT = tT.reshape(NH, P, M1, NCH, CHT)              # [h, p, m, c, t]
        tT = np.ascontiguousarray(tT.transpose(3, 1, 0, 2, 4)).astype(NP_BF16)  # [c, p, h, m, t]
        in_maps.append(
            {
                "textT": tT,
                "gnn": np.asarray(gnn_features[b], dtype=np.float32).astype(NP_BF16),
                "coords": np.asarray(seq_to_coords[b]).astype(np.float32),
                "maskf": np.ascontiguousarray(
                    np.asarray(seq_loss_mask[b]).astype(np.float32).reshape(NT, P).T
                ),
                "ls": np.full((P, 1), lsv, dtype=np.float32),
            }
        )
    return in_maps


def combine_outputs(results):
    loss = 0.0
    num = 0.0
    den = 0.0
    for r in results:
        o = np.asarray(r["out"], dtype=np.float64).reshape(4)
        loss += o[0] / o[2]
        num += o[1]
        den += o[2]
    loss = np.float32(loss / B)
    acc = np.float32(num / den)
    return np.array(loss, dtype=np.float32), np.array(acc, dtype=np.float32)


def kernel(gnn_features, text_features, logit_scale, seq_to_coords, seq_loss_mask):
    global LAST_RESULTS
    nc = get_nc()
    in_maps = make_in_maps(gnn_features, text_features, logit_scale, seq_to_coords, seq_loss_mask)
    res = run_bass_kernel_spmd(nc, in_maps, core_ids=list(range(NCORES)))
    LAST_RESULTS = res
    return combine_outputs(res.results)


# revision 9
# speedup vs baseline: 2.7822x; 2.7822x over previous
"""CLIPMutationLoss forward on 8 Trainium2 NeuronCores (data-parallel over batch).

Per core b: scores[m, t] = logit_scale * dot(text[b*20+m, t, :], gnn[b, coords[b, t], :])
loss = mean_b( sum_t mask*CE0(scores) / sum_t mask ),  acc = global masked argmax==0 rate.

Device pipeline (per core, all heavy data bf16):
  - text slab host-cast to bf16, laid out [4 chunks, 128 part, 2 d-halves, 20 m, 256 t]:
    one contiguous 2.62 MB DMA per chunk (10.5 MB total).
  - gather gnn[coords] WITHOUT indirect DMA: one-hot matmul on PE.
      O_nt[p, t] = (coords[t] == nt*128 + p)   (16 DVE tensor_scalar is_equal ops, fp16)
      selT[h][d, t] = sum_nt gnn_tile[nt].T @ O_nt   (64 bf16 matmuls, fp32 PSUM -> exact)
    This lands sel directly in [d, t] layout — no transposes, no gpsimd descriptors.
  - DVE: P[h] = textT_tile * selT_bcast  (bf16 2x mode; d on partitions, (m, t) on free)
  - PE: scores = ones-vector matmul reduction over d, both halves accumulated into
    PSUM [128 t-in-tile, 160 = (8 tt) x (20 m)] columns.
  - ACT/DVE epilogue (fp32): scale by logit_scale, log-softmax over m, class-0 CE,
    argmax==0 (exact: correct <=> s0 >= max), masked sums, partition-reduce matmul.
  - Output per core: [loss_masked_sum, correct_masked_sum, mask_sum, 0]; host combines.

bf16 error was validated against the exact seeded inputs: loss rel err ~5e-5 and the
masked accuracy is bit-identical.
"""

import numpy as np

import concourse.bacc as bacc
import concourse.bass as bass
import concourse.tile as tile
from concourse import mybir
from concourse.bass_interp import get_hw_module
from concourse.bass_utils import run_bass_kernel_spmd

B, N_NODES, D = 8, 2048, 256
T = 1024
M1 = 20  # num_mutations + 1 classes
NCORES = 8
P = 128
NCH = 4            # token chunks per core
CHT = T // NCH     # 256 tokens per chunk
NT = T // P        # 8 token tiles of 128
NH = D // P        # 2 d-halves
NNT = N_NODES // P  # 16 gnn node tiles
TH = 512           # selT build granularity (max moving free dim)
F32 = mybir.dt.float32
F16 = mybir.dt.float16
BF16 = mybir.dt.bfloat16
I32 = mybir.dt.int32
NP_BF16 = mybir.dt.np(BF16)

_NC_CACHE = {}
LAST_RESULTS = None  # test harness reads exec_time_ns off this


def _build_nc():
    nc = bacc.Bacc("TRN2", target_bir_lowering=False, debug=False)
    textT = nc.dram_tensor("textT", [NCH, P, NH, M1, CHT], BF16, kind="ExternalInput").ap()
    gnn = nc.dram_tensor("gnn", [N_NODES, D], BF16, kind="ExternalInput").ap()
    coords = nc.dram_tensor("coords", [T], F32, kind="ExternalInput").ap()
    maskf = nc.dram_tensor("maskf", [P, NT], F32, kind="ExternalInput").ap()
    ls = nc.dram_tensor("ls", [P, 1], F32, kind="ExternalInput").ap()
    out = nc.dram_tensor("out", [4, 1], F32, kind="ExternalOutput").ap()

    with (
        tile.TileContext(nc) as tc,
        tc.tile_pool(name="consts", bufs=1) as consts,
        tc.tile_pool(name="textp", bufs=3) as textp,
        tc.tile_pool(name="pp", bufs=4) as pp,
        tc.tile_pool(name="soft", bufs=1) as soft,
        tc.tile_pool(name="ps", bufs=1, space="PSUM") as ps,
        tc.tile_pool(name="gps", bufs=2, space="PSUM") as gps,
    ):
        ones_bf = consts.tile([P, 1], BF16)
        nc.vector.memset(ones_bf[:], 1.0)
        ones_f = consts.tile([P, 1], F32)
        nc.vector.memset(ones_f[:], 1.0)
        maskf_sb = consts.tile([P, NT], F32)
        nc.sync.dma_start(out=maskf_sb[:], in_=maskf[:])
        ls_sb = consts.tile([P, 1], F32)
        nc.sync.dma_start(out=ls_sb[:], in_=ls[:])

        # Touch Exp/Ln once at kernel start so the ACT table load (~2.7us) hides
        # under the initial text DMAs instead of landing on the critical tail.
        dummy = consts.tile([P, 1], F32)
        nc.scalar.activation(out=dummy[:], in_=ones_f[:], func=mybir.ActivationFunctionType.Exp)
        nc.scalar.activation(out=dummy[:], in_=dummy[:], func=mybir.ActivationFunctionType.Ln)

        # ---- one-hot gather of gnn rows on PE ----
        # coords broadcast to every partition; gnn/coords ride the SWDGE ring so
        # the HWDGE rings stream text from t=0.
        coords_all = consts.tile([P, T], F32)
        coords_b = bass.AP(tensor=coords.tensor, offset=coords.offset, ap=[[0, P], coords.ap[0]])
        nc.gpsimd.dma_start(out=coords_all[:], in_=coords_b)
        # gnn as 16 node tiles: gnn_sb[p, nt, d] = gnn[nt*128 + p, d]
        gnn_sb = consts.tile([P, NNT, D], BF16)
        nc.sync.dma_start(
            out=gnn_sb[:],
            in_=gnn.rearrange("(nt p) d -> p nt d", p=P),
        )
        # nvals[p, nt] = nt*128 + p
        nvals_i = consts.tile([P, NNT], I32)
        nc.gpsimd.iota(nvals_i[:], pattern=[[P, NNT]], base=0, channel_multiplier=1)
        nvals = consts.tile([P, NNT], F32)
        nc.vector.tensor_copy(out=nvals[:], in_=nvals_i[:])
        # O[p, nt, t] = (coords[t] == nt*128 + p), bf16 one-hot. Built per th-half
        # (tokens 0-511 first) and split across DVE + GpSimd to shorten the head.
        o_all = consts.tile([P, NNT, T], BF16)
        selT = [consts.tile([P, T], BF16, name=f"selT{h}") for h in range(NH)]
        for th in range(T // TH):
            tsl = slice(th * TH, (th + 1) * TH)
            for nt in range(NNT):
                nc.vector.tensor_scalar(
                    out=o_all[:, nt, tsl],
                    in0=coords_all[:, tsl],
                    scalar1=nvals[:, nt : nt + 1],
                    scalar2=None,
                    op0=mybir.AluOpType.is_equal,
                )
            # selT[h][d, t] = gnn[coords[t], h*128 + d] (exact bf16 via fp32 PSUM)
            for h in range(NH):
                sel_ps = gps.tile([P, TH], F32, name="sel_ps")
                for nt in range(NNT):
                    nc.tensor.matmul(
                        out=sel_ps[:],
                        lhsT=gnn_sb[:, nt, h * P : (h + 1) * P],
                        rhs=o_all[:, nt, tsl],
                        start=(nt == 0),
                        stop=(nt == NNT - 1),
                    )
                nc.scalar.copy(out=selT[h][:, tsl], in_=sel_ps[:])

        # ---- per-token class scores ----
        # columns: col = tt*20 + m  (tt = c*2 + tl, token = tt*128 + p)
        scores_ps = ps.tile([P, NT * M1], F32, name="scores_ps")
        for c in range(NCH):
            tx = textp.tile([P, NH, M1, CHT], BF16, name="tx")
            dma_eng = nc.sync if c % 2 == 0 else nc.scalar
            dma_eng.dma_start(out=tx[:], in_=textT[c])
            ptiles = []
            for h in range(NH):
                pt = pp.tile([P, M1, CHT], BF16, name="pt")
                sl = selT[h][:, c * CHT : (c + 1) * CHT]
                sl_b = bass.AP(tensor=sl.tensor, offset=sl.offset, ap=[sl.ap[0], [0, M1], sl.ap[1]])
                nc.vector.tensor_tensor(out=pt[:], in0=tx[:, h], in1=sl_b, op=mybir.AluOpType.mult)
                ptiles.append(pt)
            for g in range(2 * M1):
                tl, m = divmod(g, M1)
                col = c * 2 * M1 + g
                for h in range(NH):
                    nc.tensor.matmul(
                        out=scores_ps[:, col : col + 1],
                        lhsT=ptiles[h][:, m, tl * P : (tl + 1) * P],
                        rhs=ones_bf[:],
                        start=(h == 0),
                        stop=(h == NH - 1),
                    )

        # ---- epilogue: log-softmax over m, class-0 CE, accuracy, masked sums ----
        scores_sb = soft.tile([P, NT, M1], F32)
        nc.scalar.activation(
            out=scores_sb[:].rearrange("p t m -> p (t m)"),
            in_=scores_ps[:],
            func=mybir.ActivationFunctionType.Copy,
            bias=0.0,
            scale=ls_sb[:, 0:1],
        )
        mx = soft.tile([P, NT], F32)
        nc.vector.reduce_max(out=mx[:], in_=scores_sb[:], axis=mybir.AxisListType.X)
        sub = soft.tile([P, NT, M1], F32)
        mx_b = bass.AP(tensor=mx.tensor, offset=mx[:].offset, ap=[mx[:].ap[0], [1, NT], [0, M1]])
        nc.vector.tensor_tensor(out=sub[:], in0=scores_sb[:], in1=mx_b, op=mybir.AluOpType.subtract)
        expt = soft.tile([P, NT, M1], F32)
        nc.scalar.activation(out=expt[:], in_=sub[:], func=mybir.ActivationFunctionType.Exp)
        se = soft.tile([P, NT], F32)
        nc.vector.reduce_sum(out=se[:], in_=expt[:], axis=mybir.AxisListType.X)
        lse = soft.tile([P, NT], F32)
        nc.scalar.activation(out=lse[:], in_=se[:], func=mybir.ActivationFunctionType.Ln)

        s0 = bass.AP(tensor=scores_sb.tensor, offset=scores_sb[:].offset, ap=[scores_sb[:].ap[0], [M1, NT]])
        tmp = soft.tile([P, NT], F32)
        nc.vector.tensor_add(out=tmp[:], in0=mx[:], in1=lse[:])
        ltok = soft.tile([P, NT], F32)
        nc.vector.tensor_tensor(out=ltok[:], in0=tmp[:], in1=s0, op=mybir.AluOpType.subtract)
        corr = soft.tile([P, NT], F32)
        nc.vector.tensor_tensor(out=corr[:], in0=s0, in1=mx[:], op=mybir.AluOpType.is_ge)

        ml = soft.tile([P, NT], F32)
        nc.vector.tensor_mul(out=ml[:], in0=ltok[:], in1=maskf_sb[:])
        mc = soft.tile([P, NT], F32)
        nc.vector.tensor_mul(out=mc[:], in0=corr[:], in1=maskf_sb[:])

        stats = soft.tile([P, 4], F32)
        nc.vector.memset(stats[:], 0.0)
        nc.vector.reduce_sum(out=stats[:, 0:1], in_=ml[:], axis=mybir.AxisListType.X)
        nc.vector.reduce_sum(out=stats[:, 1:2], in_=mc[:], axis=mybir.AxisListType.X)
        nc.vector.reduce_sum(out=stats[:, 2:3], in_=maskf_sb[:], axis=mybir.AxisListType.X)

        stat_ps = ps.tile([4, 1], F32, name="stat_ps")
        nc.tensor.matmul(out=stat_ps[:], lhsT=stats[:], rhs=ones_f[:], start=True, stop=True)
        out_sb = soft.tile([4, 1], F32)
        nc.scalar.copy(out=out_sb[:], in_=stat_ps[:])
        nc.sync.dma_start(out=out[:], in_=out_sb[:])

    nc.compile()
    nc.m = get_hw_module(nc.m)
    return nc


def get_nc():
    if "nc" not in _NC_CACHE:
        _NC_CACHE["nc"] = _build_nc()
    return _NC_CACHE["nc"]


def make_in_maps(gnn_features, text_features, logit_scale, seq_to_coords, seq_loss_mask):
    in_maps = []
    lsv = np.float32(np.asarray(logit_scale).reshape(-1)[0])
    for b in range(NCORES):
        slab = np.asarray(text_features[b * M1 : (b + 1) * M1], dtype=np.float32)  # [20, 1024, 256]
        tT = slab.transpose(2, 0, 1)                      # [256 d, 20 m, 1024 t]
        tT = tT.reshape(NH, P, M1, NCH, CHT)              # [h, p, m, c, t]
        tT = np.ascontiguousarray(tT.transpose(3, 1, 0, 2, 4)).astype(NP_BF16)  # [c, p, h, m, t]
        in_maps.append(
            {
                "textT": tT,
                "gnn": np.asarray(gnn_features[b], dtype=np.float32).astype(NP_BF16),
                "coords": np.asarray(seq_to_coords[b]).astype(np.float32),
                "maskf": np.ascontiguousarray(
                    np.asarray(seq_loss_mask[b]).astype(np.float32).reshape(NT, P).T
                ),
                "ls": np.full((P, 1), lsv, dtype=np.float32),
            }
        )
    return in_maps


def combine_outputs(results):
    loss = 0.0
    num = 0.0
    den = 0.0
    for r in results:
        o = np.asarray(r["out"], dtype=np.float64).reshape(4)
        loss += o[0] / o[2]
        num += o[1]
        den += o[2]
    loss = np.float32(loss / B)
    acc = np.float32(num / den)
    return np.array(loss, dtype=np.float32), np.array(acc, dtype=np.float32)


def kernel(gnn_features, text_features, logit_scale, seq_to_coords, seq_loss_mask):
    global LAST_RESULTS
    nc = get_nc()
    in_maps = make_in_maps(gnn_features, text_features, logit_scale, seq_to_coords, seq_loss_mask)
    res = run_bass_kernel_spmd(nc, in_maps, core_ids=list(range(NCORES)))
    LAST_RESULTS = res
    return combine_outputs(res.results)
